# revision 13
# baseline (speedup 1.0000x reference)
"""DiffPool (2x GCNConv + softmax assign + S^T Z / S^T A S) on 8 Trainium2
NeuronCores via Bass/Tile.

Strategy (all FP compute on device):
  * Both GCN convs share one width-64 sparse aggregation: Ahat @ (x W) =
    (Ahat @ x) W, so we aggregate xd1 = [dinv*x | dinv | 0pad] (width 128)
    once per target node and project afterwards (bias handled exactly via the
    r = Ahat @ 1 column).
  * Sparse scatter-adds are done as one-hot matmuls: edges are host-bucketed
    by their owner 128-node window; gathered operand rows (dma_gather) are
    contracted against a one-hot lhsT built on-chip with a single is_equal
    DVE op per window, accumulating in PSUM.
  * deg (source histogram) uses the same one-hot matmuls against a ones
    vector; dinv is all-gathered.
  * S (softmax assignments) is computed node-sharded, all-gathered into a
    [50176, 512] padded table, then A@S gathers 2KB rows per edge.
  * next_X / next_A accumulate in PSUM across windows; 1.1MB all-reduce at
    the end.

Node numbering: node v belongs to core c = v // 6250; local window
w = (v % 6250) // 128, slot p = (v % 6250) % 128. Padded table row
t(v) = 6272*c + 128*w + p  (window 48 of each core has 106 valid slots).
int16 gather indices: tables are split at row 32768 into two gather calls
with rebased indices.
"""
import sys

sys.path.insert(0, "/opt/trn_rl_repo")

import numpy as np

import concourse.bass as bass
import concourse.bacc as bacc
import concourse.tile as tile
from concourse import mybir
from concourse.bass_utils import run_bass_kernel_spmd

F32 = mybir.dt.float32
I16 = mybir.dt.int16
AX = mybir.AxisListType
ALU = mybir.AluOpType
ACT_F = mybir.ActivationFunctionType

# ---------------- problem constants (hardcoded per contract) ----------------
N, E, D, K, C = 50000, 800000, 64, 500, 64
NCORES = 8
N_LOC = N // NCORES            # 6250
WPC = (N_LOC + 127) // 128     # 49 windows per core
NPAD = WPC * 128               # 6272 padded nodes per core
LAST_VALID = N_LOC - (WPC - 1) * 128   # 106 valid rows in window 48
TROWS = NCORES * NPAD          # 50176 table rows
KP = 512                       # padded cluster dim (elem 2048B)
SPLIT = 32768                  # int16 table split


# ---------------- walrus workaround: <=1 sem wait per instruction ----------
def _split_multiwait(nc, max_waits=1):
    n_split = 0
    for fn in nc.m.functions:
        for bb in fn.blocks:
            insts = bb.instructions
            new_list = []
            changed = False
            for inst in insts:
                si = inst.sync_info
                waits = list(si.on_wait) if si is not None else []
                if len(waits) > max_waits:
                    keep = waits[-max_waits:]
                    for w in waits[: len(waits) - max_waits]:
                        nop = mybir.InstNoOp(
                            name=f"{inst.name}-ws{n_split}",
                            engine=inst.engine,
                            sync_info=mybir.SyncInfo(on_wait=[w], on_update=[]),
                        )
                        new_list.append(nop)
                        n_split += 1
                    inst.sync_info = mybir.SyncInfo(
                        on_wait=keep, on_update=list(si.on_update)
                    )
                    changed = True
                new_list.append(inst)
            if changed:
                insts.clear()
                insts.extend(new_list)
    return n_split


# ---------------- host-side edge bucketing ----------------
def _tmap(v):
    c, l = np.divmod(v, N_LOC)
    w, p = np.divmod(l, 128)
    return c * NPAD + w * 128 + p


def _bucketize(scatter_v, gather_v):
    """Bucket edges by owner window of scatter_v; gather indices from the
    padded table via t(gather_v), split at SPLIT.

    Returns (idx16 [NC,WPC,128,(WA+WB)//16], relf [NC,WPC,128,(WA+WB)//128],
             WA, WB).
    """
    t = _tmap(gather_v).astype(np.int64)
    own = scatter_v // N_LOC
    loc = scatter_v % N_LOC
    win = loc // 128
    rel = (loc % 128).astype(np.float32)
    gw = own * WPC + win                     # global window id
    half = (t >= SPLIT).astype(np.int64)     # 0=A, 1=B

    key = gw * 2 + half
    order = np.argsort(key, kind="stable")
    t_s, rel_s, key_s = t[order], rel[order], key[order]
    counts = np.bincount(key_s, minlength=NCORES * WPC * 2)
    cA = counts[0::2].reshape(NCORES, WPC)
    cB = counts[1::2].reshape(NCORES, WPC)
    WA = max(128, int(-(-cA.max() // 128) * 128))
    WB = max(128, int(-(-cB.max() // 128) * 128))
    W = WA + WB

    idxpad = np.zeros((NCORES, WPC, W), np.int64)
    relpad = np.full((NCORES, WPC, W), -1.0, np.float32)
    starts = np.concatenate([[0], np.cumsum(counts)])
    for c in range(NCORES):
        for w in range(WPC):
            kA = (c * WPC + w) * 2
            sA, eA = starts[kA], starts[kA + 1]
            nA = eA - sA
            idxpad[c, w, :nA] = t_s[sA:eA]
            relpad[c, w, :nA] = rel_s[sA:eA]
            sB, eB = starts[kA + 1], starts[kA + 2]
            nB = eB - sB
            idxpad[c, w, WA : WA + nB] = t_s[sB:eB] - SPLIT
            relpad[c, w, WA : WA + nB] = rel_s[sB:eB]

    # wrap idx halves into the 16-partition layout, replicate to 128
    def wrap(a, width):  # a: [NC,WPC,width]
        a = a.reshape(NCORES, WPC, width // 16, 16).transpose(0, 1, 3, 2)
        return np.tile(a, (1, 1, 8, 1))      # [NC,WPC,128,width//16]

    idx16 = np.concatenate(
        [wrap(idxpad[:, :, :WA], WA), wrap(idxpad[:, :, WA:], WB)], axis=3
    ).astype(np.int16)
    # rel into [128 partitions, chunk] layout: slot i -> [i%128, i//128]
    relf = relpad.reshape(NCORES, WPC, W // 128, 128).transpose(0, 1, 3, 2).copy()
    return idx16, relf, WA, WB


MAX_GATHER = 1024  # SWDGE descriptor ring holds 1024 descriptors


def _emit_gathers(nc, dst, tbl_lo, tbl_hi, idxt, WA, WB, elem):
    """Gather half A (slots [0,WA) from tbl_lo) and half B (slots [WA,WA+WB)
    from tbl_hi) into dst [128, (WA+WB)//128, elem], splitting calls at
    MAX_GATHER indices (ring capacity)."""
    for base, width, tbl in ((0, WA, tbl_lo), (WA, WB, tbl_hi)):
        off = 0
        while off < width:
            L = min(MAX_GATHER, width - off)
            s = base + off
            nc.gpsimd.dma_gather(
                dst[:, s // 128 : (s + L) // 128, :], tbl,
                idxt[:, s // 16 : (s + L) // 16], L, L, elem,
            )
            off += L


# ---------------- device program ----------------
def _build(cfg, max_phase=7):
    WAg, WBg, WAs, WBs = cfg
    NCHG = (WAg + WBg) // 128
    CHAG = WAg // 128
    NCHS = (WAs + WBs) // 128
    CHAS = WAs // 128

    nc = bacc.Bacc("TRN2", target_bir_lowering=False, debug=False,
                   num_devices=NCORES)

    def inp(name, shape, dtype=F32):
        return nc.dram_tensor(name, shape, dtype, kind="ExternalInput")

    x_full = inp("x", [N, D])
    w_e = inp("W_embed", [D, C])
    b_e = inp("b_embed", [C])
    w_a = inp("W_assign", [D, K])
    b_a = inp("b_assign", [K])
    x_loc = inp("x_loc", [NPAD, D])
    iota_c = inp("iota", [128, 128])
    ident_c = inp("ident", [128, 128])
    ones_c = inp("ones", [128, 1])
    vmask_c = inp("vmask", [128, 1])   # 1.0 for p < LAST_VALID else 0.0
    gidx = inp("gidx", [WPC, 128, (WAg + WBg) // 16], I16)
    grel = inp("grel", [WPC, 128, NCHG])
    sidx = inp("sidx", [WPC, 128, (WAs + WBs) // 16], I16)
    srel = inp("srel", [WPC, 128, NCHS])

    out_X = nc.dram_tensor("next_X", [K, C], F32, kind="ExternalOutput")
    out_A = nc.dram_tensor("next_A", [K, K], F32, kind="ExternalOutput")

    xd1_dram = nc.dram_tensor("xd1_tbl", [TROWS, 128], F32)
    s_loc = nc.dram_tensor("s_loc", [NPAD, KP], F32)
    z_loc = nc.dram_tensor("z_loc", [NPAD, C], F32)
    dinv_loc = nc.dram_tensor("dinv_loc", [128, WPC], F32)
    dinv_all = nc.dram_tensor("dinv_all", [NCORES, 128, WPC], F32,
                              addr_space="Shared")
    s_all = nc.dram_tensor("s_all", [TROWS, KP], F32, addr_space="Shared")
    nx_bounce = nc.dram_tensor("nx_bounce", [4, 128, C], F32)
    nx_red = nc.dram_tensor("nx_red", [4, 128, C], F32, addr_space="Shared")
    na_bounce = nc.dram_tensor("na_bounce", [4, 128, KP], F32)
    na_red = nc.dram_tensor("na_red", [4, 128, KP], F32, addr_space="Shared")

    groups = [list(range(NCORES))]

    with tile.TileContext(nc) as tc:
        with (
            tc.tile_pool(name="const", bufs=1) as constp,
            tc.tile_pool(name="persist", bufs=1) as perp,
        ):
            iota_t = constp.tile([128, 128], F32)
            nc.sync.dma_start(iota_t[:], iota_c[:])
            ident_t = constp.tile([128, 128], F32)
            nc.sync.dma_start(ident_t[:], ident_c[:])
            ones_t = constp.tile([128, 1], F32)
            nc.sync.dma_start(ones_t[:], ones_c[:])
            vmask_t = constp.tile([128, 1], F32)
            nc.sync.dma_start(vmask_t[:], vmask_c[:])
            we1 = constp.tile([65, C], F32)
            nc.sync.dma_start(we1[0:64, :], w_e[:])
            nc.sync.dma_start(we1[64:65, :], b_e[:].unsqueeze(0))
            wa1 = constp.tile([65, K], F32)
            nc.sync.dma_start(wa1[0:64, :], w_a[:])
            nc.sync.dma_start(wa1[64:65, :], b_a[:].unsqueeze(0))

            deg_sb = perp.tile([128, WPC], F32)
            dinv_sb = perp.tile([128, WPC], F32)
            dinv2_sb = perp.tile([128, WPC], F32)

            # ---------------- phase 1: deg histogram + dinv -------------
            _P = max_phase
            with (
                tc.tile_pool(name="p1", bufs=3) as p1,
                tc.tile_pool(name="p1ps", bufs=2, space="PSUM") as p1ps,
            ):
                for w in range(WPC):
                    relt = p1.tile([128, NCHS], F32, tag="rel")
                    nc.sync.dma_start(relt[:], srel[w])
                    oh = p1.tile([128, NCHS, 128], F32, tag="oh")
                    nc.vector.tensor_tensor(
                        oh[:],
                        iota_t[:].unsqueeze(1).broadcast_to([128, NCHS, 128]),
                        relt[:].unsqueeze(2).broadcast_to([128, NCHS, 128]),
                        ALU.is_equal,
                    )
                    dg = p1ps.tile([128, 1], F32, tag="deg")
                    for ch in range(NCHS):
                        nc.tensor.matmul(
                            dg[:], oh[:, ch, :], ones_t[:],
                            start=(ch == 0), stop=(ch == NCHS - 1),
                        )
                    nc.vector.tensor_copy(deg_sb[:, w : w + 1], dg[:])
                # dinv = 1/sqrt(deg+1); dinv2 = dinv^2
                sq = p1.tile([128, WPC], F32)
                nc.scalar.activation(sq[:], deg_sb[:], ACT_F.Sqrt, bias=1.0)
                nc.vector.reciprocal(dinv_sb[:], sq[:])
                nc.vector.tensor_tensor(dinv2_sb[:], dinv_sb[:], dinv_sb[:],
                                        ALU.mult)
                nc.sync.dma_start(dinv_loc[:], dinv_sb[:])
                nc.gpsimd.collective_compute(
                    "AllGather", ALU.bypass, replica_groups=groups,
                    ins=[dinv_loc[:]], outs=[dinv_all[:]],
                )

            dinv_all_sb = perp.tile([128, NCORES, WPC], F32)
            nc.sync.dma_start(
                dinv_all_sb[:], dinv_all.ap().rearrange("c p w -> p c w")
            )
            dinv_flat = dinv_all_sb[:].rearrange("p c w -> p (c w)")

            # ---------------- phase 2: xd1 table build ------------------
            with tc.tile_pool(name="p2", bufs=3) as p2:
                for c8 in range(NCORES):
                    for grp in range(13):
                        if grp < 12:
                            w0, wins, rows = grp * 4, 4, 128
                        else:
                            w0, wins, rows = 48, 1, LAST_VALID
                        base = N_LOC * c8 + 128 * w0
                        W0 = WPC * c8 + w0
                        xt = p2.tile([128, wins, D], F32, tag="xt")
                        if rows < 128:
                            nc.vector.memset(xt[:], 0.0)
                        nc.sync.dma_start(
                            xt[:rows, :, :],
                            x_full[base : base + 128 * (wins - 1) + rows]
                            .rearrange("(j p) d -> p j d", p=128)
                            if rows == 128
                            else x_full[base : base + rows].unsqueeze(1),
                        )
                        xd1t = p2.tile([128, wins, 128], F32, tag="xd1")
                        nc.vector.memset(xd1t[:], 0.0)
                        dslice = dinv_flat[:, W0 : W0 + wins]
                        nc.vector.tensor_tensor(
                            xd1t[:, :, 0:D], xt[:],
                            dslice.unsqueeze(2).broadcast_to([128, wins, D]),
                            ALU.mult,
                        )
                        nc.vector.tensor_copy(
                            xd1t[:, :, D : D + 1], dslice.unsqueeze(2)
                        )
                        r0 = NPAD * c8 + 128 * w0
                        nc.sync.dma_start(
                            xd1_dram[r0 : r0 + 128 * (wins - 1) + rows]
                            .rearrange("(j p) d -> p j d", p=128)
                            if rows == 128
                            else xd1_dram[r0 : r0 + rows].unsqueeze(1),
                            xd1t[:rows, :, :],
                        )

            # ---------------- phase 3: g aggregation -> Pe --------------
            pe_all = perp.tile([128, WPC, 65], F32)
            with (
                tc.tile_pool(name="p3", bufs=2) as p3,
                tc.tile_pool(name="p3ps", bufs=2, space="PSUM") as p3ps,
            ):
                for w in range(WPC):
                    idxt = p3.tile([128, (WAg + WBg) // 16], I16, tag="idx")
                    nc.sync.dma_start(idxt[:], gidx[w])
                    relt = p3.tile([128, NCHG], F32, tag="rel")
                    nc.sync.dma_start(relt[:], grel[w])
                    gbuf = p3.tile([128, NCHG, 128], F32, tag="gbuf")
                    _emit_gathers(
                        nc, gbuf, xd1_dram[0:SPLIT], xd1_dram[SPLIT:TROWS],
                        idxt, WAg, WBg, 128,
                    )
                    oh = p3.tile([128, NCHG, 128], F32, tag="oh")
                    nc.vector.tensor_tensor(
                        oh[:],
                        iota_t[:].unsqueeze(1).broadcast_to([128, NCHG, 128]),
                        relt[:].unsqueeze(2).broadcast_to([128, NCHG, 128]),
                        ALU.is_equal,
                    )
                    agg = p3ps.tile([128, 128], F32, tag="agg")
                    for ch in range(NCHG):
                        nc.tensor.matmul(
                            agg[:], oh[:, ch, :], gbuf[:, ch, :],
                            start=(ch == 0), stop=(ch == NCHG - 1),
                        )
                    xw = p3.tile([128, D], F32, tag="xw")
                    nc.sync.dma_start(xw[:], x_loc[128 * w : 128 * (w + 1)])
                    sx = p3.tile([128, D], F32, tag="sx")
                    nc.vector.tensor_scalar(
                        sx[:], xw[:], dinv2_sb[:, w : w + 1], None, ALU.mult
                    )
                    t1 = p3.tile([128, 65], F32, tag="t1")
                    nc.vector.tensor_scalar(
                        t1[:], agg[:, 0:65], dinv_sb[:, w : w + 1], None,
                        ALU.mult,
                    )
                    nc.vector.tensor_tensor(
                        pe_all[:, w, 0:D], t1[:, 0:D], sx[:], ALU.add
                    )
                    nc.vector.tensor_tensor(
                        pe_all[:, w, D : D + 1], t1[:, D : D + 1],
                        dinv2_sb[:, w : w + 1], ALU.add,
                    )

            # ---------------- phase 4: project + softmax + next_X -------
            with (
                tc.tile_pool(name="p4", bufs=3) as p4,
                tc.tile_pool(name="p4pt", bufs=1, space="PSUM") as p4pt,
                tc.tile_pool(name="p4z", bufs=1, space="PSUM") as p4z,
                tc.tile_pool(name="p4l", bufs=2, space="PSUM") as p4l,
                tc.tile_pool(name="p4nx", bufs=1, space="PSUM") as p4nx,
            ):
                nx = [p4nx.tile([128, C], F32, tag=f"nx{s}", name=f"nx{s}")
                      for s in range(4)]
                for w in range(WPC):
                    ptp = p4pt.tile([65, 128], F32, tag="pt")
                    nc.tensor.transpose(ptp[:], pe_all[:, w, :], ident_t[:])
                    pts = p4.tile([65, 128], F32, tag="pts")
                    nc.vector.tensor_copy(pts[:], ptp[:])
                    zp = p4z.tile([128, C], F32, tag="zp")
                    nc.tensor.matmul(zp[:], pts[:], we1[:])
                    zsb = p4.tile([128, C], F32, tag="zsb")
                    nc.vector.tensor_copy(zsb[:], zp[:])
                    nc.sync.dma_start(z_loc[128 * w : 128 * (w + 1)], zsb[:])
                    lp = p4l.tile([128, K], F32, tag="lp")
                    nc.tensor.matmul(lp[:], pts[:], wa1[:])
                    negmax = p4.tile([128, 1], F32, tag="negmax")
                    nc.vector.reduce_max(negmax[:], lp[:], AX.X, negate=True)
                    ex = p4.tile([128, K], F32, tag="ex")
                    ssum = p4.tile([128, 1], F32, tag="ssum")
                    nc.scalar.activation(
                        ex[:], lp[:], ACT_F.Exp, bias=negmax[:], scale=1.0,
                        accum_out=ssum[:],
                    )
                    rsum = p4.tile([128, 1], F32, tag="rsum")
                    nc.vector.reciprocal(rsum[:], ssum[:])
                    if w == WPC - 1:
                        # zero the 22 pad rows by masking the softmax scale
                        nc.vector.tensor_tensor(
                            rsum[:], rsum[:], vmask_t[:], ALU.mult
                        )
                    sw = p4.tile([128, KP], F32, tag="sw")
                    nc.vector.memset(sw[:, K:KP], 0.0)
                    nc.scalar.activation(
                        sw[:, 0:K], ex[:], ACT_F.Copy, bias=0.0, scale=rsum[:]
                    )
                    for s in range(4):
                        nc.tensor.matmul(
                            nx[s][:], sw[:, 128 * s : 128 * (s + 1)], zsb[:],
                            start=(w == 0), stop=(w == WPC - 1),
                        )
                    nc.sync.dma_start(s_loc[128 * w : 128 * (w + 1)], sw[:])
                nxsb = p4.tile([128, 4, C], F32, tag="nxsb")
                for s in range(4):
                    nc.vector.tensor_copy(nxsb[:, s, :], nx[s][:])
                nc.sync.dma_start(
                    nx_bounce.ap().rearrange("s p d -> p s d"), nxsb[:]
                )
                nc.gpsimd.collective_compute(
                    "AllReduce", ALU.add, replica_groups=groups,
                    ins=[nx_bounce[:]], outs=[nx_red[:]],
                )

            # ---------------- phase 5: S all-gather ---------------------
            nc.gpsimd.collective_compute(
                "AllGather", ALU.bypass, replica_groups=groups,
                ins=[s_loc[:]], outs=[s_all[:]],
            )

            # ---------------- phase 6: AS aggregation + next_A ----------
            with (
                tc.tile_pool(name="p6", bufs=2) as p6,
                tc.tile_pool(name="p6ps", bufs=2, space="PSUM") as p6ps,
                tc.tile_pool(name="p6na", bufs=1, space="PSUM") as p6na,
            ):
                na = [p6na.tile([128, KP], F32, tag=f"na{s}", name=f"na{s}")
                      for s in range(4)]
                for w in range(WPC):
                    idxt = p6.tile([128, (WAs + WBs) // 16], I16, tag="idx")
                    nc.sync.dma_start(idxt[:], sidx[w])
                    relt = p6.tile([128, NCHS], F32, tag="rel")
                    nc.sync.dma_start(relt[:], srel[w])
                    sg = p6.tile([128, NCHS, KP], F32, tag="sg")
                    _emit_gathers(
                        nc, sg, s_all[0:SPLIT], s_all[SPLIT:TROWS],
                        idxt, WAs, WBs, KP,
                    )
                    oh = p6.tile([128, NCHS, 128], F32, tag="oh")
                    nc.vector.tensor_tensor(
                        oh[:],
                        iota_t[:].unsqueeze(1).broadcast_to([128, NCHS, 128]),
                        relt[:].unsqueeze(2).broadcast_to([128, NCHS, 128]),
                        ALU.is_equal,
                    )
                    asw = p6ps.tile([128, KP], F32, tag="asw")
                    for ch in range(NCHS):
                        nc.tensor.matmul(
                            asw[:], oh[:, ch, :], sg[:, ch, :],
                            start=(ch == 0), stop=(ch == NCHS - 1),
                        )
                    asw_sb = p6.tile([128, KP], F32, tag="aswsb")
                    nc.vector.tensor_copy(asw_sb[:], asw[:])
                    ssb = p6.tile([128, KP], F32, tag="ssb")
                    nc.sync.dma_start(ssb[:], s_loc[128 * w : 128 * (w + 1)])
                    for s in range(4):
                        nc.tensor.matmul(
                            na[s][:], ssb[:, 128 * s : 128 * (s + 1)],
                            asw_sb[:],
                            start=(w == 0), stop=(w == WPC - 1),
                        )
                nasb = p6.tile([128, 4, KP], F32, tag="nasb")
                for s in range(4):
                    nc.vector.tensor_copy(nasb[:, s, :], na[s][:])
                nc.sync.dma_start(
                    na_bounce.ap().rearrange("s p d -> p s d"), nasb[:]
                )
                nc.gpsimd.collective_compute(
                    "AllReduce", ALU.add, replica_groups=groups,
                    ins=[na_bounce[:]], outs=[na_red[:]],
                )

            # ---------------- phase 7: outputs --------------------------
            nc.sync.dma_start(
                out_X[:], nx_red.ap().rearrange("s p d -> (s p) d")[0:K, :]
            )
            nc.sync.dma_start(
                out_A[:],
                na_red.ap().rearrange("s p d -> (s p) d")[0:K, 0:K],
            )

    nc.compile()
    _split_multiwait(nc)
    bass.Bass.finalize(nc)
    return nc


# ---------------- host driver ----------------
_CACHE = {}


def _get_program(cfg):
    if cfg not in _CACHE:
        _CACHE[cfg] = _build(cfg)
    return _CACHE[cfg]


def kernel(x, edge_index, W_embed, b_embed, W_assign, b_assign,
           _want_trace=False):
    x = np.asarray(x, np.float32)
    edge_index = np.asarray(edge_index, np.int32)
    W_embed = np.asarray(W_embed, np.float32)
    b_embed = np.asarray(b_embed, np.float32)
    W_assign = np.asarray(W_assign, np.float32)
    b_assign = np.asarray(b_assign, np.float32)
    row = edge_index[0].astype(np.int64)
    col = edge_index[1].astype(np.int64)

    # g phase: scatter by col (target), gather xd1[row]
    gidx, grel, WAg, WBg = _bucketize(col, row)
    # AS phase (and deg): scatter by row (source), gather S[col]
    sidx, srel, WAs, WBs = _bucketize(row, col)

    nc = _get_program((WAg, WBg, WAs, WBs))

    iota = np.tile(np.arange(128, dtype=np.float32), (128, 1))
    ident = np.eye(128, dtype=np.float32)
    ones = np.ones((128, 1), np.float32)
    vmask = (np.arange(128) < LAST_VALID).astype(np.float32).reshape(128, 1)

    in_maps = []
    for c in range(NCORES):
        xl = np.zeros((NPAD, D), np.float32)
        xl[:N_LOC] = x[c * N_LOC : (c + 1) * N_LOC]
        in_maps.append({
            "x": x, "W_embed": W_embed, "b_embed": b_embed,
            "W_assign": W_assign, "b_assign": b_assign,
            "x_loc": xl, "iota": iota, "ident": ident, "ones": ones,
            "vmask": vmask,
            "gidx": gidx[c], "grel": grel[c],
            "sidx": sidx[c], "srel": srel[c],
        })

    if _want_trace:
        out, ns = _run_timed(nc, in_maps)
        kernel._last_exec_ns = ns
    else:
        res = run_bass_kernel_spmd(nc, in_maps, list(range(NCORES)))
        out = res.results[0]
    return out["next_X"].copy(), out["next_A"].copy()


def _run_timed(nc, in_maps, iters=4):
    """Mirror bass2jax.run_bass_via_pjrt's multi-core path, but pre-place
    inputs on the devices and wall-clock repeated executions (min over
    iters). Returns (core0 outputs dict, best_ns)."""
    import time

    import jax
    from jax.sharding import Mesh, NamedSharding, PartitionSpec
    from jax.experimental.shard_map import shard_map

    from concourse import bass2jax, mybir as mb

    bass2jax.install_neuronx_cc_hook()
    n_cores = len(in_maps)
    in_names, out_names, out_avals, zero_outs = [], [], [], []
    partition_name = (nc.partition_id_tensor.name
                      if nc.partition_id_tensor else None)
    for alloc in nc.m.functions[0].allocations:
        if not isinstance(alloc, mb.MemoryLocationSet):
            continue
        name = alloc.memorylocations[0].name
        if alloc.kind == "ExternalInput":
            if name != partition_name:
                in_names.append(name)
        elif alloc.kind == "ExternalOutput":
            shape = tuple(alloc.tensor_shape)
            dtype = mb.dt.np(alloc.dtype)
            out_names.append(name)
            out_avals.append(jax.core.ShapedArray(shape, dtype))
            zero_outs.append(np.zeros(shape, dtype))
    n_params = len(in_names)
    n_outs = len(out_avals)
    all_in_names = list(in_names) + list(out_names)
    if partition_name is not None:
        all_in_names.append(partition_name)

    def _body(*args):
        operands = list(args)
        if partition_name is not None:
            operands.append(bass2jax.partition_id_tensor())
        outs = bass2jax._bass_exec_p.bind(
            *operands,
            out_avals=tuple(out_avals),
            in_names=tuple(all_in_names),
            out_names=tuple(out_names),
            lowering_input_output_aliases=(),
            sim_require_finite=True,
            sim_require_nnan=True,
            nc=nc,
        )
        return tuple(outs)

    devices = jax.devices()[:n_cores]
    mesh = Mesh(np.asarray(devices), ("core",))
    in_specs = (PartitionSpec("core"),) * (n_params + n_outs)
    out_specs = (PartitionSpec("core"),) * len(out_names)
    sharded = jax.jit(
        shard_map(_body, mesh=mesh, in_specs=in_specs, out_specs=out_specs,
                  check_rep=False),
        keep_unused=True,
    )
    sh = NamedSharding(mesh, PartitionSpec("core"))
    concat_in = [
        jax.device_put(
            np.concatenate([np.asarray(in_maps[c][nm])
                            for c in range(n_cores)], axis=0), sh)
        for nm in in_names
    ]
    concat_zeros = [
        jax.device_put(np.zeros((n_cores * z.shape[0], *z.shape[1:]), z.dtype),
                       sh)
        for z in zero_outs
    ]
    jax.block_until_ready(concat_in)
    jax.block_until_ready(concat_zeros)
    best = None
    out_arrs = None
    for _ in range(iters):
        t0 = time.perf_counter()
        out_arrs = sharded(*concat_in, *concat_zeros)
        jax.block_until_ready(out_arrs)
        dt = time.perf_counter() - t0
        best = dt if best is None else min(best, dt)
    core0 = {
        name: np.asarray(out_arrs[i]).reshape(n_cores, *out_avals[i].shape)[0]
        for i, name in enumerate(out_names)
    }
    return core0, int(best * 1e9)
